# revision 23
# baseline (speedup 1.0000x reference)
"""Trainium2 Bass kernel for nn_DifferentialRenderLoss.

Algorithm: the volume-render trilinear gather is separable per depth sample
(rays are axis-aligned: R == I).  For depth sample k the rendered frame is
  out = A_k @ ((1-fz) vol[z0] + fz vol[z1]) @ B_k^T
with A_k [PH,VH], B_k [PW,VW] sparse tent-weight matrices (<=2 nnz/row).
Only samples whose z lies inside the volume contribute (~4 of 200); the
rest multiply exact 1.0 factors into the raymarch and add exact 0.0 to the
sums, so skipping them is lossless.  The z-blend folds into A (two
PSUM-accumulated matmuls).  Each of the 8 cores renders an 18-pixel-wide
column stripe of all cameras (loading only the volume W-slices it needs),
does the raymarch + Huber losses for its stripe, plus a W-chunk of the BEV
reduction, and writes partial sums; the host combines them.
"""
import sys

if "/opt/trn_rl_repo" not in sys.path:
    sys.path.insert(0, "/opt/trn_rl_repo")

import numpy as np

# ---- problem configuration (mirrors the nn.Module init_kwargs) ----
N_CAM = 2
PH, PW = 96, 144
NPTS = 200
MIN_DEPTH, MAX_DEPTH = 1.0, 4000.0
VD, VH, VW = 32, 128, 384
VOXEL = 2.5
VOL_TRANS = np.zeros(3, np.float32)
CH = 4                      # density + rgb
N_CORES = 8
PCW = PW // N_CORES         # pixel columns per core
WB = VW // N_CORES          # bev W-chunk per core


# ---------------------------------------------------------------- host math
def _tent_matrix(g, n):
    """Dense interpolation matrix mirroring the reference's floor/frac +
    per-corner mask + clip arithmetic bitwise (all float32)."""
    P = g.shape[0]
    A = np.zeros((P, n), np.float32)
    g0 = np.floor(g)
    f = (g - g0).astype(np.float32)
    i0 = g0.astype(np.int32)
    rows = np.arange(P)
    for d, w in ((0, (np.float32(1.0) - f).astype(np.float32)), (1, f)):
        idx = i0 + d
        valid = (idx >= 0) & (idx < n)
        np.add.at(A, (rows, np.clip(idx, 0, n - 1)),
                  np.where(valid, w, np.float32(0.0)).astype(np.float32))
    return A


def _plan(focal, principal, R, T):
    """Per-camera active depth samples with tent matrices (float32 host math
    mirroring the reference)."""
    focal = np.asarray(focal, np.float32)
    principal = np.asarray(principal, np.float32)
    R = np.asarray(R, np.float32)
    T = np.asarray(T, np.float32)
    xs = np.arange(PW, dtype=np.float32) + np.float32(0.5)
    ys = np.arange(PH, dtype=np.float32) + np.float32(0.5)
    depths = np.linspace(MIN_DEPTH, MAX_DEPTH, NPTS, dtype=np.float32)
    half = np.array([VOXEL * (VW - 1) / 2.0, VOXEL * (VH - 1) / 2.0,
                     VOXEL * (VD - 1) / 2.0], np.float32)
    cams = []
    for c in range(N_CAM):
        if not np.allclose(R[c], np.eye(3), atol=1e-6):
            raise NotImplementedError("kernel fast path requires R == I")
        dir_x = ((xs - principal[c, 0]) / focal[c, 0]).astype(np.float32)
        dir_y = ((ys - principal[c, 1]) / focal[c, 1]).astype(np.float32)
        origin = (-(T[c] @ R[c].T)).astype(np.float32)
        samples = []
        for k in range(NPTS):
            t = depths[k]
            zw = np.float32(origin[2] + t)          # dir_z == 1
            gz = np.float32((np.float32((zw - VOL_TRANS[2]) / half[2]) + np.float32(1.0))
                            * np.float32(0.5) * (VD - 1))
            if gz <= -1.0 or gz >= VD:
                continue
            z0 = int(np.floor(gz))
            fz = np.float32(gz - np.floor(gz))
            wz0 = np.float32(1.0) - fz if z0 >= 0 else np.float32(0.0)
            wz1 = fz if z0 + 1 <= VD - 1 else np.float32(0.0)
            gy = ((((origin[1] + t * dir_y) - VOL_TRANS[1]) / half[1]
                   + np.float32(1.0)) * np.float32(0.5) * (VH - 1)).astype(np.float32)
            gx = ((((origin[0] + t * dir_x) - VOL_TRANS[0]) / half[0]
                   + np.float32(1.0)) * np.float32(0.5) * (VW - 1)).astype(np.float32)
            A = _tent_matrix(gy, VH)
            B = _tent_matrix(gx, VW)
            if not (A.any() and B.any() and (wz0 or wz1)):
                continue
            samples.append(dict(k=k, z0=min(max(z0, 0), VD - 1),
                                z1=min(max(z0 + 1, 0), VD - 1),
                                wz0=wz0, wz1=wz1, A=A, B=B))
        cams.append(samples)
    return cams


def _dedup(cams):
    """Group cameras with identical plans. Returns (unique_plans, cam2u)."""
    uniq, cam2u = [], []
    for s in cams:
        found = None
        for ui, u in enumerate(uniq):
            if len(u) == len(s) and all(
                a["k"] == b["k"] and a["z0"] == b["z0"] and a["z1"] == b["z1"]
                and a["wz0"] == b["wz0"] and a["wz1"] == b["wz1"]
                and np.array_equal(a["A"], b["A"]) and np.array_equal(a["B"], b["B"])
                for a, b in zip(u, s)
            ):
                found = ui
                break
        if found is None:
            uniq.append(list(s))
            cam2u.append(len(uniq) - 1)
        else:
            cam2u.append(found)
    return uniq, cam2u


def _pad_plans(uniq):
    """Pad every unique plan to a common NK with all-zero dummy samples
    (zero A/B/wz => exact zero density, raymarch unaffected)."""
    nk = max([len(u) for u in uniq] + [1])
    zero = dict(k=-1, z0=0, z1=0, wz0=np.float32(0), wz1=np.float32(0),
                A=np.zeros((PH, VH), np.float32),
                B=np.zeros((PW, VW), np.float32))
    for u in uniq:
        while len(u) < nk:
            u.append(zero)
    return nk


# ------------------------------------------------------------ device program
_PROG_CACHE = {}


def _build_program(NU, NK, WC, cam2u):
    import concourse.bacc as bacc
    import concourse.mybir as mybir
    import concourse.tile as tile

    F32 = mybir.dt.float32
    AF = mybir.ActivationFunctionType
    AX = mybir.AxisListType
    OP = mybir.AluOpType

    nc = bacc.Bacc(None)
    # head = at | bt | vol(k=0) ; vol2 = vol(k>0)  (fewer DMA issues)
    AT_COLS = NU * NK * PH
    BT_COLS = NU * NK * PCW
    V0_COLS = NU * CH * WC
    head_d = nc.dram_tensor("headpack", [128, AT_COLS + BT_COLS + V0_COLS], F32,
                            kind="ExternalInput")
    vol2_d = nc.dram_tensor("vol2pack", [128, max(1, NU * (NK - 1) * CH * WC)], F32,
                            kind="ExternalInput")
    tgt_d = nc.dram_tensor("tgtpack", [PH, N_CAM * 4 * PCW], F32,
                           kind="ExternalInput")
    bev_d = nc.dram_tensor("bevpack", [128, WB * VD], F32, kind="ExternalInput")
    out_d = nc.dram_tensor("out", [1, 16], F32, kind="ExternalOutput")

    with tile.TileContext(nc) as tc:
        with tc.tile_pool(name="sb", bufs=1) as sb, \
             tc.tile_pool(name="sb2", bufs=2) as sb2, \
             tc.tile_pool(name="ps1", bufs=2, space="PSUM") as ps1, \
             tc.tile_pool(name="ps2", bufs=2, space="PSUM") as ps2, \
             tc.tile_pool(name="ps3", bufs=1, space="PSUM") as ps3:

            head_sb = sb.tile([128, AT_COLS + BT_COLS + V0_COLS], F32)
            vol2_sb = sb.tile([128, max(1, NU * (NK - 1) * CH * WC)], F32)
            tgt_sb = sb.tile([PH, N_CAM * 4 * PCW], F32)
            bev_sb = sb.tile([128, WB * VD], F32)
            at_sb = head_sb[:, 0:AT_COLS]
            bt_sb = head_sb[0:WC, AT_COLS:AT_COLS + BT_COLS]

            # warm the ACT tables (Copy + Sqrt) while DMAs stream
            pack = sb.tile([128, 16], F32)
            nc.gpsimd.memset(pack[:], 0.0)
            ones = sb.tile([128, 1], F32)
            nc.gpsimd.memset(ones[:], 1.0)
            warm = sb.tile([1, 1], F32)
            nc.scalar.activation(warm[:], ones[0:1, :], AF.Sqrt,
                                 bias=1.0, scale=100.0)
            warm2 = sb.tile([1, 1], F32)
            nc.scalar.copy(out=warm2[:], in_=ones[0:1, :])

            nc.sync.dma_start(out=head_sb[:], in_=head_d[:])
            nc.sync.dma_start(out=vol2_sb[:], in_=vol2_d[:])
            nc.sync.dma_start(out=tgt_sb[:], in_=tgt_d[:])
            nc.sync.dma_start(out=bev_sb[:], in_=bev_d[:])

            def vol_slice(u, k, ch):
                if k == 0:
                    off = AT_COLS + BT_COLS + (u * CH + ch) * WC
                    return head_sb[:, off:off + WC]
                off = (u * (NK - 1) + (k - 1)) * CH * WC + ch * WC
                return vol2_sb[:, off:off + WC]

            # ---- render, software-pipelined on PE ----
            # step1(u,k): y_ps [WC, CH*PH] = 4 matmuls (per ch)
            # step2(u,k): p_ps [PH, CH*PCW] = 4 matmuls, transposed output
            #             (rows on partitions: cheap raymarch ops)
            y_sbs = {}
            p_ps_t = {}
            P = [[None] * NK for _ in range(NU)]

            def step1(u, k):
                uk = u * NK + k
                y_ps = ps1.tile([WC, CH * PH], F32, tag="y")
                aoff = uk * PH
                for ch in range(CH):
                    nc.tensor.matmul(
                        y_ps[:, ch * PH:(ch + 1) * PH],
                        lhsT=vol_slice(u, k, ch),
                        rhs=at_sb[:, aoff:aoff + PH],
                        start=True, stop=True)
                y_sb = sb2.tile([WC, CH * PH], F32, tag="ysb")
                nc.scalar.copy(out=y_sb[:], in_=y_ps[:])
                y_sbs[(u, k)] = y_sb

            def step2(u, k):
                uk = u * NK + k
                y_sb = y_sbs[(u, k)]
                p_ps = ps2.tile([PH, CH * PCW], F32, tag="p")
                for ch in range(CH):
                    nc.tensor.matmul(
                        p_ps[:, ch * PCW:(ch + 1) * PCW],
                        lhsT=y_sb[:, ch * PH:(ch + 1) * PH],
                        rhs=bt_sb[:, uk * PCW:(uk + 1) * PCW],
                        start=True, stop=True)
                p_sb = sb.tile([PH, CH * PCW], F32, tag=f"p_{u}_{k}")
                nc.scalar.copy(out=p_sb[:], in_=p_ps[:])
                P[u][k] = p_sb

            units = [(u, k) for u in range(NU) for k in range(NK)]
            step1(*units[0])
            for i in range(1, len(units)):
                step1(*units[i])
                step2(*units[i - 1])
            step2(*units[-1])

            # ---- raymarch per unique cam; rend[u] [PH, 4*PCW] matches tgt ----
            rend = [None] * NU
            for u in range(NU):
                r_t = sb.tile([PH, 4 * PCW], F32, tag=f"rend{u}")
                f3 = r_t[:, PCW:4 * PCW].rearrange("p (c f) -> p c f", c=3)
                d0 = P[u][0][:, 0:PCW]
                d0b = d0.rearrange("p (o f) -> p o f", o=1) \
                    .to_broadcast([PH, 3, PCW])
                rgb0 = P[u][0][:, PCW:4 * PCW].rearrange("p (c f) -> p c f", c=3)
                nc.vector.tensor_mul(f3, d0b, rgb0)
                shifted = sb.tile([PH, PCW], F32, tag=f"sh{u}_0")
                nc.vector.tensor_scalar(shifted[:], d0, -1.0, 1.0, OP.mult, OP.add)
                for k in range(1, NK):
                    dk = P[u][k][:, 0:PCW]
                    rgbk = P[u][k][:, PCW:4 * PCW].rearrange("p (c f) -> p c f", c=3)
                    w_t = sb.tile([PH, PCW], F32, tag=f"w{u}_{k}")
                    nc.vector.tensor_mul(w_t[:], dk, shifted[:])
                    wb = w_t[:].rearrange("p (o f) -> p o f", o=1) \
                        .to_broadcast([PH, 3, PCW])
                    t_t = sb.tile([PH, 3 * PCW], F32, tag=f"t{u}_{k}")
                    t3 = t_t[:].rearrange("p (c f) -> p c f", c=3)
                    nc.vector.tensor_mul(t3, wb, rgbk)
                    nc.vector.tensor_add(f3, f3, t3)
                    sh_new = sb.tile([PH, PCW], F32, tag=f"sh{u}_{k}")
                    # shifted*(1-dk) == shifted - shifted*dk == shifted - w
                    nc.vector.tensor_sub(sh_new[:], shifted[:], w_t[:])
                    shifted = sh_new
                nc.vector.tensor_scalar(r_t[:, 0:PCW], shifted[:], -1.0, 1.0,
                                        OP.mult, OP.add)
                rend[u] = r_t

            # ---- Huber losses per camera: accumulate sum(sqrt(1+100*d^2)) ----
            # pack cols: cam*2 = sil sum, cam*2+1 = color sum (3ch), 15 = bev
            for cam in range(N_CAM):
                u = cam2u[cam]
                diff = sb.tile([PH, 4 * PCW], F32, tag=f"diff{cam}")
                nc.vector.tensor_sub(diff[:], rend[u][:],
                                     tgt_sb[:, cam * 4 * PCW:(cam + 1) * 4 * PCW])
                sq = sb.tile([PH, 4 * PCW], F32, tag=f"sq{cam}")
                nc.vector.tensor_mul(sq[:], diff[:], diff[:])
                hub_s = sb.tile([PH, PCW], F32, tag=f"hub_s{cam}")
                nc.scalar.activation(hub_s[:], sq[:, 0:PCW], AF.Sqrt,
                                     bias=1.0, scale=100.0,
                                     accum_out=pack[0:PH, cam * 2:cam * 2 + 1])
                hub_c = sb.tile([PH, 3 * PCW], F32, tag=f"hub_c{cam}")
                nc.scalar.activation(hub_c[:], sq[:, PCW:4 * PCW], AF.Sqrt,
                                     bias=1.0, scale=100.0,
                                     accum_out=pack[0:PH, cam * 2 + 1:cam * 2 + 2])

            # ---- bev: sum over (h,w-chunk) of |max_d density| (2 halves) ----
            bmax = sb.tile([128, WB], F32)
            hw = WB // 2
            for h in range(2):
                nc.vector.reduce_max(
                    bmax[:, h * hw:(h + 1) * hw],
                    bev_sb[:, h * hw * VD:(h + 1) * hw * VD]
                    .rearrange("p (w d) -> p w d", d=VD),
                    axis=AX.X)
            nc.vector.tensor_reduce(pack[:, 15:16], bmax[:], axis=AX.X, op=OP.add,
                                    apply_absolute_value=True)

            # ---- cross-partition reduction via ones-matmul ----
            out_ps = ps3.tile([1, 16], F32)
            nc.tensor.matmul(out_ps[:], lhsT=ones[:], rhs=pack[:],
                             start=True, stop=True)
            out_sb = sb.tile([1, 16], F32)
            nc.scalar.copy(out=out_sb[:], in_=out_ps[:])
            nc.sync.dma_start(out=out_d[:], in_=out_sb[:])

    nc.compile()
    return nc


# ------------------------------------------------------------- host packing
def _pack_core(core, uniq, NK, WC, vol, dens, tsil, timg):
    NU = len(uniq)
    qlo = core * PCW
    qhi = qlo + PCW
    # union W-range over all (u,k) for this core's pixel columns
    wlo = VW
    whi = 0
    for u in uniq:
        for s in u:
            cols = np.nonzero(s["B"][qlo:qhi].any(axis=0))[0]
            if cols.size:
                wlo = min(wlo, int(cols[0]))
                whi = max(whi, int(cols[-1]) + 1)
    if wlo >= whi:
        wlo, whi = 0, 1
    if whi - wlo > WC:
        raise AssertionError(f"core {core}: W-range {whi - wlo} > WC {WC}")
    wlo = min(wlo, VW - 1)
    span = min(WC, VW - wlo)

    AT_COLS = NU * NK * PH
    BT_COLS = NU * NK * PCW
    V0_COLS = NU * CH * WC
    headpack = np.zeros((128, AT_COLS + BT_COLS + V0_COLS), np.float32)
    vol2pack = np.zeros((128, max(1, NU * (NK - 1) * CH * WC)), np.float32)
    for ui, u in enumerate(uniq):
        for k, s in enumerate(u):
            uk = ui * NK + k
            # z-preblended volume chunk for this (u,k)
            for ch in range(CH):
                blend = (s["wz0"] * vol[ch, s["z0"], :, wlo:wlo + span]
                         + s["wz1"] * vol[ch, s["z1"], :, wlo:wlo + span]) \
                    .astype(np.float32)
                if k == 0:
                    off = AT_COLS + BT_COLS + (ui * CH + ch) * WC
                    headpack[:, off:off + span] = blend
                else:
                    off = (ui * (NK - 1) + (k - 1)) * CH * WC + ch * WC
                    vol2pack[:, off:off + span] = blend
            headpack[:, uk * PH:(uk + 1) * PH] = s["A"].T
            headpack[:span, AT_COLS + uk * PCW:AT_COLS + (uk + 1) * PCW] = \
                s["B"][qlo:qhi, wlo:wlo + span].T
    tgtpack = np.zeros((PH, N_CAM * 4 * PCW), np.float32)
    for cam in range(N_CAM):
        tgtpack[:, (cam * 4) * PCW:(cam * 4 + 1) * PCW] = tsil[cam, :, qlo:qhi]
        for c in range(3):
            tgtpack[:, (cam * 4 + 1 + c) * PCW:(cam * 4 + 2 + c) * PCW] = \
                timg[cam, :, qlo:qhi, c]
    blo = core * WB
    bevpack = np.ascontiguousarray(
        dens[:, :, blo:blo + WB].transpose(1, 2, 0)).reshape(128, WB * VD)
    return dict(headpack=headpack, vol2pack=vol2pack,
                tgtpack=tgtpack, bevpack=np.ascontiguousarray(bevpack))


def _compute_wc(uniq):
    """Max W-range width over all cores, padded to a multiple of 4."""
    wc = 1
    for core in range(N_CORES):
        qlo, qhi = core * PCW, (core + 1) * PCW
        wlo, whi = VW, 0
        for u in uniq:
            for s in u:
                cols = np.nonzero(s["B"][qlo:qhi].any(axis=0))[0]
                if cols.size:
                    wlo = min(wlo, int(cols[0]))
                    whi = max(whi, int(cols[-1]) + 1)
        if wlo < whi:
            wc = max(wc, whi - wlo)
    wc = min(-(-wc // 4) * 4, 128)
    return wc


# ------------------------------------------------------------------- kernel
_RUN_MODE = "hw"     # "hw" | "sim" (CoreSim, debugging only)


def _run(nc, in_maps):
    if _RUN_MODE == "sim":
        from concourse.bass_interp import CoreSim

        class R:
            results = []
        for m in in_maps:
            sim = CoreSim(nc)
            for name, arr in m.items():
                sim.tensor(name)[:] = arr
            sim.simulate()
            R.results.append({"out": np.array(sim.tensor("out"))})
        return R
    from concourse.bass_utils import run_bass_kernel_spmd
    res = run_bass_kernel_spmd(nc, in_maps, list(range(N_CORES)))
    global _LAST_RESULT
    _LAST_RESULT = res
    return res


_LAST_RESULT = None


def kernel(densities, colors, target_silhouettes, target_images,
           focal, principal, R, T):

    densities = np.asarray(densities, np.float32)
    colors = np.asarray(colors, np.float32)
    tsil = np.asarray(target_silhouettes, np.float32)
    timg = np.asarray(target_images, np.float32)

    cams = _plan(focal, principal, R, T)
    uniq, cam2u = _dedup(cams)
    NK = _pad_plans(uniq)
    NU = len(uniq)
    WC = _compute_wc(uniq)
    assert WC <= 128, f"WC={WC} exceeds PE stationary width"

    key = (NU, NK, WC, tuple(cam2u))
    if key not in _PROG_CACHE:
        _PROG_CACHE[key] = _build_program(NU, NK, WC, cam2u)
    nc = _PROG_CACHE[key]

    vol = np.concatenate([densities[0], colors[0]], axis=0)  # [4,VD,VH,VW]
    dens = densities[0, 0]                                    # [VD,VH,VW]
    in_maps = [_pack_core(c, uniq, NK, WC, vol, dens, tsil, timg)
               for c in range(N_CORES)]
    res = _run(nc, in_maps)

    sil_S = 0.0
    col_S = 0.0
    bev_S = 0.0
    for c in range(N_CORES):
        o = res.results[c]["out"][0]
        for cam in range(N_CAM):
            sil_S += float(o[cam * 2])
            col_S += float(o[cam * 2 + 1])
        bev_S += float(o[15])
    n_sil = N_CAM * PH * PW
    n_col = N_CAM * PH * PW * 3
    sil_err = np.float32(0.1 * (sil_S - n_sil) / n_sil)
    col_err = np.float32(0.1 * (col_S - n_col) / n_col)
    bev_err = np.float32(bev_S / (VH * VW))
    return (col_err, sil_err, bev_err)


# revision 24
# speedup vs baseline: 1.0464x; 1.0464x over previous
"""Trainium2 Bass kernel for nn_DifferentialRenderLoss.

Algorithm: the volume-render trilinear gather is separable per depth sample
(rays are axis-aligned: R == I).  For depth sample k the rendered frame is
  out = A_k @ ((1-fz) vol[z0] + fz vol[z1]) @ B_k^T
with A_k [PH,VH], B_k [PW,VW] sparse tent-weight matrices (<=2 nnz/row).
Only samples whose z lies inside the volume contribute (~4 of 200); the
rest multiply exact 1.0 factors into the raymarch and add exact 0.0 to the
sums, so skipping them is lossless.  The z-blend folds into A (two
PSUM-accumulated matmuls).  Each of the 8 cores renders an 18-pixel-wide
column stripe of all cameras (loading only the volume W-slices it needs),
does the raymarch + Huber losses for its stripe, plus a W-chunk of the BEV
reduction, and writes partial sums; the host combines them.
"""
import sys

if "/opt/trn_rl_repo" not in sys.path:
    sys.path.insert(0, "/opt/trn_rl_repo")

import numpy as np

# ---- problem configuration (mirrors the nn.Module init_kwargs) ----
N_CAM = 2
PH, PW = 96, 144
NPTS = 200
MIN_DEPTH, MAX_DEPTH = 1.0, 4000.0
VD, VH, VW = 32, 128, 384
VOXEL = 2.5
VOL_TRANS = np.zeros(3, np.float32)
CH = 4                      # density + rgb
N_CORES = 8
PCW = PW // N_CORES         # pixel columns per core
WB = VW // N_CORES          # bev W-chunk per core


# ---------------------------------------------------------------- host math
def _tent_matrix(g, n):
    """Dense interpolation matrix mirroring the reference's floor/frac +
    per-corner mask + clip arithmetic bitwise (all float32)."""
    P = g.shape[0]
    A = np.zeros((P, n), np.float32)
    g0 = np.floor(g)
    f = (g - g0).astype(np.float32)
    i0 = g0.astype(np.int32)
    rows = np.arange(P)
    for d, w in ((0, (np.float32(1.0) - f).astype(np.float32)), (1, f)):
        idx = i0 + d
        valid = (idx >= 0) & (idx < n)
        np.add.at(A, (rows, np.clip(idx, 0, n - 1)),
                  np.where(valid, w, np.float32(0.0)).astype(np.float32))
    return A


def _plan(focal, principal, R, T):
    """Per-camera active depth samples with tent matrices (float32 host math
    mirroring the reference)."""
    focal = np.asarray(focal, np.float32)
    principal = np.asarray(principal, np.float32)
    R = np.asarray(R, np.float32)
    T = np.asarray(T, np.float32)
    xs = np.arange(PW, dtype=np.float32) + np.float32(0.5)
    ys = np.arange(PH, dtype=np.float32) + np.float32(0.5)
    depths = np.linspace(MIN_DEPTH, MAX_DEPTH, NPTS, dtype=np.float32)
    half = np.array([VOXEL * (VW - 1) / 2.0, VOXEL * (VH - 1) / 2.0,
                     VOXEL * (VD - 1) / 2.0], np.float32)
    cams = []
    for c in range(N_CAM):
        if not np.allclose(R[c], np.eye(3), atol=1e-6):
            raise NotImplementedError("kernel fast path requires R == I")
        dir_x = ((xs - principal[c, 0]) / focal[c, 0]).astype(np.float32)
        dir_y = ((ys - principal[c, 1]) / focal[c, 1]).astype(np.float32)
        origin = (-(T[c] @ R[c].T)).astype(np.float32)
        samples = []
        for k in range(NPTS):
            t = depths[k]
            zw = np.float32(origin[2] + t)          # dir_z == 1
            gz = np.float32((np.float32((zw - VOL_TRANS[2]) / half[2]) + np.float32(1.0))
                            * np.float32(0.5) * (VD - 1))
            if gz <= -1.0 or gz >= VD:
                continue
            z0 = int(np.floor(gz))
            fz = np.float32(gz - np.floor(gz))
            wz0 = np.float32(1.0) - fz if z0 >= 0 else np.float32(0.0)
            wz1 = fz if z0 + 1 <= VD - 1 else np.float32(0.0)
            gy = ((((origin[1] + t * dir_y) - VOL_TRANS[1]) / half[1]
                   + np.float32(1.0)) * np.float32(0.5) * (VH - 1)).astype(np.float32)
            gx = ((((origin[0] + t * dir_x) - VOL_TRANS[0]) / half[0]
                   + np.float32(1.0)) * np.float32(0.5) * (VW - 1)).astype(np.float32)
            A = _tent_matrix(gy, VH)
            B = _tent_matrix(gx, VW)
            if not (A.any() and B.any() and (wz0 or wz1)):
                continue
            samples.append(dict(k=k, z0=min(max(z0, 0), VD - 1),
                                z1=min(max(z0 + 1, 0), VD - 1),
                                wz0=wz0, wz1=wz1, A=A, B=B))
        cams.append(samples)
    return cams


def _dedup(cams):
    """Group cameras with identical plans. Returns (unique_plans, cam2u)."""
    uniq, cam2u = [], []
    for s in cams:
        found = None
        for ui, u in enumerate(uniq):
            if len(u) == len(s) and all(
                a["k"] == b["k"] and a["z0"] == b["z0"] and a["z1"] == b["z1"]
                and a["wz0"] == b["wz0"] and a["wz1"] == b["wz1"]
                and np.array_equal(a["A"], b["A"]) and np.array_equal(a["B"], b["B"])
                for a, b in zip(u, s)
            ):
                found = ui
                break
        if found is None:
            uniq.append(list(s))
            cam2u.append(len(uniq) - 1)
        else:
            cam2u.append(found)
    return uniq, cam2u


def _pad_plans(uniq):
    """Pad every unique plan to a common NK with all-zero dummy samples
    (zero A/B/wz => exact zero density, raymarch unaffected)."""
    nk = max([len(u) for u in uniq] + [1])
    zero = dict(k=-1, z0=0, z1=0, wz0=np.float32(0), wz1=np.float32(0),
                A=np.zeros((PH, VH), np.float32),
                B=np.zeros((PW, VW), np.float32))
    for u in uniq:
        while len(u) < nk:
            u.append(zero)
    return nk


# ------------------------------------------------------------ device program
_PROG_CACHE = {}


def _build_program(NU, NK, WC, cam2u):
    import concourse.bacc as bacc
    import concourse.mybir as mybir
    import concourse.tile as tile

    F32 = mybir.dt.float32
    AF = mybir.ActivationFunctionType
    AX = mybir.AxisListType
    OP = mybir.AluOpType

    nc = bacc.Bacc(None)
    # head = at | bt | vol(k=0) ; vol2 = vol(k>0)  (fewer DMA issues)
    AT_COLS = NU * NK * PH
    BT_COLS = NU * NK * PCW
    V0_COLS = NU * CH * WC
    head_d = nc.dram_tensor("headpack", [128, AT_COLS + BT_COLS + V0_COLS], F32,
                            kind="ExternalInput")
    vol2_d = nc.dram_tensor("vol2pack", [128, max(1, NU * (NK - 1) * CH * WC)], F32,
                            kind="ExternalInput")
    tgt_d = nc.dram_tensor("tgtpack", [PH, N_CAM * 4 * PCW], F32,
                           kind="ExternalInput")
    bev_d = nc.dram_tensor("bevpack", [128, WB * VD], F32, kind="ExternalInput")
    out_d = nc.dram_tensor("out", [1, 16], F32, kind="ExternalOutput")

    with tile.TileContext(nc) as tc:
        with tc.tile_pool(name="sb", bufs=1) as sb, \
             tc.tile_pool(name="sb2", bufs=3) as sb2, \
             tc.tile_pool(name="ps1", bufs=3, space="PSUM") as ps1, \
             tc.tile_pool(name="ps2", bufs=3, space="PSUM") as ps2, \
             tc.tile_pool(name="ps3", bufs=1, space="PSUM") as ps3:

            head_sb = sb.tile([128, AT_COLS + BT_COLS + V0_COLS], F32)
            vol2_sb = sb.tile([128, max(1, NU * (NK - 1) * CH * WC)], F32)
            tgt_sb = sb.tile([PH, N_CAM * 4 * PCW], F32)
            bev_sb = sb.tile([128, WB * VD], F32)
            at_sb = head_sb[:, 0:AT_COLS]
            bt_sb = head_sb[0:WC, AT_COLS:AT_COLS + BT_COLS]

            # warm the ACT tables (Copy + Sqrt) while DMAs stream
            pack = sb.tile([128, 16], F32)
            nc.gpsimd.memset(pack[:], 0.0)
            ones = sb.tile([128, 1], F32)
            nc.gpsimd.memset(ones[:], 1.0)
            warm = sb.tile([1, 1], F32)
            nc.scalar.activation(warm[:], ones[0:1, :], AF.Sqrt,
                                 bias=1.0, scale=100.0)
            warm2 = sb.tile([1, 1], F32)
            nc.scalar.copy(out=warm2[:], in_=ones[0:1, :])
            warm_src = sb.tile([128, 512], F32)
            nc.gpsimd.memset(warm_src[:], 1.0)
            warm_ps = ps3.tile([1, 512], F32, tag="warmps")
            for _ in range(5):
                nc.tensor.matmul(warm_ps[:], lhsT=ones[:], rhs=warm_src[:],
                                 start=True, stop=True)

            nc.sync.dma_start(out=head_sb[:], in_=head_d[:])
            if NK >= 3:
                c1 = CH * WC
                nc.sync.dma_start(out=vol2_sb[:, :c1], in_=vol2_d[:, :c1])
                nc.sync.dma_start(out=vol2_sb[:, c1:], in_=vol2_d[:, c1:])
            else:
                nc.sync.dma_start(out=vol2_sb[:], in_=vol2_d[:])
            nc.sync.dma_start(out=tgt_sb[:], in_=tgt_d[:])
            nc.sync.dma_start(out=bev_sb[:], in_=bev_d[:])

            def vol_slice(u, k, ch):
                if k == 0:
                    off = AT_COLS + BT_COLS + (u * CH + ch) * WC
                    return head_sb[:, off:off + WC]
                off = (u * (NK - 1) + (k - 1)) * CH * WC + ch * WC
                return vol2_sb[:, off:off + WC]

            # ---- render, software-pipelined on PE ----
            # step1(u,k): y_ps [WC, CH*PH] = 4 matmuls (per ch)
            # step2(u,k): p_ps [PH, CH*PCW] = 4 matmuls, transposed output
            #             (rows on partitions: cheap raymarch ops)
            y_sbs = {}
            p_ps_t = {}
            P = [[None] * NK for _ in range(NU)]

            def step1(u, k):
                uk = u * NK + k
                y_ps = ps1.tile([WC, CH * PH], F32, tag="y")
                aoff = uk * PH
                for ch in range(CH):
                    nc.tensor.matmul(
                        y_ps[:, ch * PH:(ch + 1) * PH],
                        lhsT=vol_slice(u, k, ch),
                        rhs=at_sb[:, aoff:aoff + PH],
                        start=True, stop=True)
                y_sb = sb2.tile([WC, CH * PH], F32, tag="ysb")
                nc.scalar.copy(out=y_sb[:], in_=y_ps[:])
                y_sbs[(u, k)] = y_sb

            def step2(u, k):
                uk = u * NK + k
                y_sb = y_sbs[(u, k)]
                p_ps = ps2.tile([PH, CH * PCW], F32, tag="p")
                for ch in range(CH):
                    nc.tensor.matmul(
                        p_ps[:, ch * PCW:(ch + 1) * PCW],
                        lhsT=y_sb[:, ch * PH:(ch + 1) * PH],
                        rhs=bt_sb[:, uk * PCW:(uk + 1) * PCW],
                        start=True, stop=True)
                p_sb = sb.tile([PH, CH * PCW], F32, tag=f"p_{u}_{k}")
                nc.scalar.copy(out=p_sb[:], in_=p_ps[:])
                P[u][k] = p_sb

            units = [(u, k) for u in range(NU) for k in range(NK)]
            step1(*units[0])
            for i in range(1, len(units)):
                step1(*units[i])
                step2(*units[i - 1])
            step2(*units[-1])

            # ---- raymarch per unique cam; rend[u] [PH, 4*PCW] matches tgt ----
            rend = [None] * NU
            for u in range(NU):
                r_t = sb.tile([PH, 4 * PCW], F32, tag=f"rend{u}")
                f3 = r_t[:, PCW:4 * PCW].rearrange("p (c f) -> p c f", c=3)
                d0 = P[u][0][:, 0:PCW]
                d0b = d0.rearrange("p (o f) -> p o f", o=1) \
                    .to_broadcast([PH, 3, PCW])
                rgb0 = P[u][0][:, PCW:4 * PCW].rearrange("p (c f) -> p c f", c=3)
                nc.vector.tensor_mul(f3, d0b, rgb0)
                shifted = sb.tile([PH, PCW], F32, tag=f"sh{u}_0")
                nc.vector.tensor_scalar(shifted[:], d0, -1.0, 1.0, OP.mult, OP.add)
                for k in range(1, NK):
                    dk = P[u][k][:, 0:PCW]
                    rgbk = P[u][k][:, PCW:4 * PCW].rearrange("p (c f) -> p c f", c=3)
                    w_t = sb.tile([PH, PCW], F32, tag=f"w{u}_{k}")
                    nc.vector.tensor_mul(w_t[:], dk, shifted[:])
                    wb = w_t[:].rearrange("p (o f) -> p o f", o=1) \
                        .to_broadcast([PH, 3, PCW])
                    t_t = sb.tile([PH, 3 * PCW], F32, tag=f"t{u}_{k}")
                    t3 = t_t[:].rearrange("p (c f) -> p c f", c=3)
                    nc.vector.tensor_mul(t3, wb, rgbk)
                    nc.vector.tensor_add(f3, f3, t3)
                    sh_new = sb.tile([PH, PCW], F32, tag=f"sh{u}_{k}")
                    # shifted*(1-dk) == shifted - shifted*dk == shifted - w
                    nc.vector.tensor_sub(sh_new[:], shifted[:], w_t[:])
                    shifted = sh_new
                nc.vector.tensor_scalar(r_t[:, 0:PCW], shifted[:], -1.0, 1.0,
                                        OP.mult, OP.add)
                rend[u] = r_t

            # ---- Huber losses per camera: accumulate sum(sqrt(1+100*d^2)) ----
            # pack cols: cam*2 = sil sum, cam*2+1 = color sum (3ch), 15 = bev
            for cam in range(N_CAM):
                u = cam2u[cam]
                diff = sb.tile([PH, 4 * PCW], F32, tag=f"diff{cam}")
                nc.vector.tensor_sub(diff[:], rend[u][:],
                                     tgt_sb[:, cam * 4 * PCW:(cam + 1) * 4 * PCW])
                sq = sb.tile([PH, 4 * PCW], F32, tag=f"sq{cam}")
                nc.vector.tensor_mul(sq[:], diff[:], diff[:])
                hub_s = sb.tile([PH, PCW], F32, tag=f"hub_s{cam}")
                nc.scalar.activation(hub_s[:], sq[:, 0:PCW], AF.Sqrt,
                                     bias=1.0, scale=100.0,
                                     accum_out=pack[0:PH, cam * 2:cam * 2 + 1])
                hub_c = sb.tile([PH, 3 * PCW], F32, tag=f"hub_c{cam}")
                nc.scalar.activation(hub_c[:], sq[:, PCW:4 * PCW], AF.Sqrt,
                                     bias=1.0, scale=100.0,
                                     accum_out=pack[0:PH, cam * 2 + 1:cam * 2 + 2])

            # ---- bev: sum over (h,w-chunk) of |max_d density| (2 halves) ----
            bmax = sb.tile([128, WB], F32)
            hw = WB // 2
            for h in range(2):
                nc.vector.reduce_max(
                    bmax[:, h * hw:(h + 1) * hw],
                    bev_sb[:, h * hw * VD:(h + 1) * hw * VD]
                    .rearrange("p (w d) -> p w d", d=VD),
                    axis=AX.X)
            nc.vector.tensor_reduce(pack[:, 15:16], bmax[:], axis=AX.X, op=OP.add,
                                    apply_absolute_value=True)

            # ---- cross-partition reduction via ones-matmul ----
            out_ps = ps3.tile([1, 16], F32)
            nc.tensor.matmul(out_ps[:], lhsT=ones[:], rhs=pack[:],
                             start=True, stop=True)
            out_sb = sb.tile([1, 16], F32)
            nc.scalar.copy(out=out_sb[:], in_=out_ps[:])
            nc.sync.dma_start(out=out_d[:], in_=out_sb[:])

    nc.compile()
    return nc


# ------------------------------------------------------------- host packing
def _pack_core(core, uniq, NK, WC, vol, dens, tsil, timg):
    NU = len(uniq)
    qlo = core * PCW
    qhi = qlo + PCW
    # union W-range over all (u,k) for this core's pixel columns
    wlo = VW
    whi = 0
    for u in uniq:
        for s in u:
            cols = np.nonzero(s["B"][qlo:qhi].any(axis=0))[0]
            if cols.size:
                wlo = min(wlo, int(cols[0]))
                whi = max(whi, int(cols[-1]) + 1)
    if wlo >= whi:
        wlo, whi = 0, 1
    if whi - wlo > WC:
        raise AssertionError(f"core {core}: W-range {whi - wlo} > WC {WC}")
    wlo = min(wlo, VW - 1)
    span = min(WC, VW - wlo)

    AT_COLS = NU * NK * PH
    BT_COLS = NU * NK * PCW
    V0_COLS = NU * CH * WC
    headpack = np.zeros((128, AT_COLS + BT_COLS + V0_COLS), np.float32)
    vol2pack = np.zeros((128, max(1, NU * (NK - 1) * CH * WC)), np.float32)
    for ui, u in enumerate(uniq):
        for k, s in enumerate(u):
            uk = ui * NK + k
            # z-preblended volume chunk for this (u,k)
            for ch in range(CH):
                blend = (s["wz0"] * vol[ch, s["z0"], :, wlo:wlo + span]
                         + s["wz1"] * vol[ch, s["z1"], :, wlo:wlo + span]) \
                    .astype(np.float32)
                if k == 0:
                    off = AT_COLS + BT_COLS + (ui * CH + ch) * WC
                    headpack[:, off:off + span] = blend
                else:
                    off = (ui * (NK - 1) + (k - 1)) * CH * WC + ch * WC
                    vol2pack[:, off:off + span] = blend
            headpack[:, uk * PH:(uk + 1) * PH] = s["A"].T
            headpack[:span, AT_COLS + uk * PCW:AT_COLS + (uk + 1) * PCW] = \
                s["B"][qlo:qhi, wlo:wlo + span].T
    tgtpack = np.zeros((PH, N_CAM * 4 * PCW), np.float32)
    for cam in range(N_CAM):
        tgtpack[:, (cam * 4) * PCW:(cam * 4 + 1) * PCW] = tsil[cam, :, qlo:qhi]
        for c in range(3):
            tgtpack[:, (cam * 4 + 1 + c) * PCW:(cam * 4 + 2 + c) * PCW] = \
                timg[cam, :, qlo:qhi, c]
    blo = core * WB
    bevpack = np.ascontiguousarray(
        dens[:, :, blo:blo + WB].transpose(1, 2, 0)).reshape(128, WB * VD)
    return dict(headpack=headpack, vol2pack=vol2pack,
                tgtpack=tgtpack, bevpack=np.ascontiguousarray(bevpack))


def _compute_wc(uniq):
    """Max W-range width over all cores, padded to a multiple of 4."""
    wc = 1
    for core in range(N_CORES):
        qlo, qhi = core * PCW, (core + 1) * PCW
        wlo, whi = VW, 0
        for u in uniq:
            for s in u:
                cols = np.nonzero(s["B"][qlo:qhi].any(axis=0))[0]
                if cols.size:
                    wlo = min(wlo, int(cols[0]))
                    whi = max(whi, int(cols[-1]) + 1)
        if wlo < whi:
            wc = max(wc, whi - wlo)
    wc = min(-(-wc // 4) * 4, 128)
    return wc


# ------------------------------------------------------------------- kernel
_RUN_MODE = "hw"     # "hw" | "sim" (CoreSim, debugging only)


def _run(nc, in_maps):
    if _RUN_MODE == "sim":
        from concourse.bass_interp import CoreSim

        class R:
            results = []
        for m in in_maps:
            sim = CoreSim(nc)
            for name, arr in m.items():
                sim.tensor(name)[:] = arr
            sim.simulate()
            R.results.append({"out": np.array(sim.tensor("out"))})
        return R
    from concourse.bass_utils import run_bass_kernel_spmd
    res = run_bass_kernel_spmd(nc, in_maps, list(range(N_CORES)))
    global _LAST_RESULT
    _LAST_RESULT = res
    return res


_LAST_RESULT = None


def kernel(densities, colors, target_silhouettes, target_images,
           focal, principal, R, T):

    densities = np.asarray(densities, np.float32)
    colors = np.asarray(colors, np.float32)
    tsil = np.asarray(target_silhouettes, np.float32)
    timg = np.asarray(target_images, np.float32)

    cams = _plan(focal, principal, R, T)
    uniq, cam2u = _dedup(cams)
    NK = _pad_plans(uniq)
    NU = len(uniq)
    WC = _compute_wc(uniq)
    assert WC <= 128, f"WC={WC} exceeds PE stationary width"

    key = (NU, NK, WC, tuple(cam2u))
    if key not in _PROG_CACHE:
        _PROG_CACHE[key] = _build_program(NU, NK, WC, cam2u)
    nc = _PROG_CACHE[key]

    vol = np.concatenate([densities[0], colors[0]], axis=0)  # [4,VD,VH,VW]
    dens = densities[0, 0]                                    # [VD,VH,VW]
    in_maps = [_pack_core(c, uniq, NK, WC, vol, dens, tsil, timg)
               for c in range(N_CORES)]
    res = _run(nc, in_maps)

    sil_S = 0.0
    col_S = 0.0
    bev_S = 0.0
    for c in range(N_CORES):
        o = res.results[c]["out"][0]
        for cam in range(N_CAM):
            sil_S += float(o[cam * 2])
            col_S += float(o[cam * 2 + 1])
        bev_S += float(o[15])
    n_sil = N_CAM * PH * PW
    n_col = N_CAM * PH * PW * 3
    sil_err = np.float32(0.1 * (sil_S - n_sil) / n_sil)
    col_err = np.float32(0.1 * (col_S - n_col) / n_col)
    bev_err = np.float32(bev_S / (VH * VW))
    return (col_err, sil_err, bev_err)


# revision 25
# speedup vs baseline: 1.0915x; 1.0431x over previous
"""Trainium2 Bass kernel for nn_DifferentialRenderLoss.

Algorithm: the volume-render trilinear gather is separable per depth sample
(rays are axis-aligned: R == I).  For depth sample k the rendered frame is
  out = A_k @ ((1-fz) vol[z0] + fz vol[z1]) @ B_k^T
with A_k [PH,VH], B_k [PW,VW] sparse tent-weight matrices (<=2 nnz/row).
Only samples whose z lies inside the volume contribute (~4 of 200); the
rest multiply exact 1.0 factors into the raymarch and add exact 0.0 to the
sums, so skipping them is lossless.  The z-blend folds into A (two
PSUM-accumulated matmuls).  Each of the 8 cores renders an 18-pixel-wide
column stripe of all cameras (loading only the volume W-slices it needs),
does the raymarch + Huber losses for its stripe, plus a W-chunk of the BEV
reduction, and writes partial sums; the host combines them.
"""
import sys

if "/opt/trn_rl_repo" not in sys.path:
    sys.path.insert(0, "/opt/trn_rl_repo")

import numpy as np

# ---- problem configuration (mirrors the nn.Module init_kwargs) ----
N_CAM = 2
PH, PW = 96, 144
NPTS = 200
MIN_DEPTH, MAX_DEPTH = 1.0, 4000.0
VD, VH, VW = 32, 128, 384
VOXEL = 2.5
VOL_TRANS = np.zeros(3, np.float32)
CH = 4                      # density + rgb
N_CORES = 8
PCW = PW // N_CORES         # pixel columns per core
WB = VW // N_CORES          # bev W-chunk per core


# ---------------------------------------------------------------- host math
def _tent_matrix(g, n):
    """Dense interpolation matrix mirroring the reference's floor/frac +
    per-corner mask + clip arithmetic bitwise (all float32)."""
    P = g.shape[0]
    A = np.zeros((P, n), np.float32)
    g0 = np.floor(g)
    f = (g - g0).astype(np.float32)
    i0 = g0.astype(np.int32)
    rows = np.arange(P)
    for d, w in ((0, (np.float32(1.0) - f).astype(np.float32)), (1, f)):
        idx = i0 + d
        valid = (idx >= 0) & (idx < n)
        np.add.at(A, (rows, np.clip(idx, 0, n - 1)),
                  np.where(valid, w, np.float32(0.0)).astype(np.float32))
    return A


def _plan(focal, principal, R, T):
    """Per-camera active depth samples with tent matrices (float32 host math
    mirroring the reference)."""
    focal = np.asarray(focal, np.float32)
    principal = np.asarray(principal, np.float32)
    R = np.asarray(R, np.float32)
    T = np.asarray(T, np.float32)
    xs = np.arange(PW, dtype=np.float32) + np.float32(0.5)
    ys = np.arange(PH, dtype=np.float32) + np.float32(0.5)
    depths = np.linspace(MIN_DEPTH, MAX_DEPTH, NPTS, dtype=np.float32)
    half = np.array([VOXEL * (VW - 1) / 2.0, VOXEL * (VH - 1) / 2.0,
                     VOXEL * (VD - 1) / 2.0], np.float32)
    cams = []
    for c in range(N_CAM):
        if not np.allclose(R[c], np.eye(3), atol=1e-6):
            raise NotImplementedError("kernel fast path requires R == I")
        dir_x = ((xs - principal[c, 0]) / focal[c, 0]).astype(np.float32)
        dir_y = ((ys - principal[c, 1]) / focal[c, 1]).astype(np.float32)
        origin = (-(T[c] @ R[c].T)).astype(np.float32)
        samples = []
        for k in range(NPTS):
            t = depths[k]
            zw = np.float32(origin[2] + t)          # dir_z == 1
            gz = np.float32((np.float32((zw - VOL_TRANS[2]) / half[2]) + np.float32(1.0))
                            * np.float32(0.5) * (VD - 1))
            if gz <= -1.0 or gz >= VD:
                continue
            z0 = int(np.floor(gz))
            fz = np.float32(gz - np.floor(gz))
            wz0 = np.float32(1.0) - fz if z0 >= 0 else np.float32(0.0)
            wz1 = fz if z0 + 1 <= VD - 1 else np.float32(0.0)
            gy = ((((origin[1] + t * dir_y) - VOL_TRANS[1]) / half[1]
                   + np.float32(1.0)) * np.float32(0.5) * (VH - 1)).astype(np.float32)
            gx = ((((origin[0] + t * dir_x) - VOL_TRANS[0]) / half[0]
                   + np.float32(1.0)) * np.float32(0.5) * (VW - 1)).astype(np.float32)
            A = _tent_matrix(gy, VH)
            B = _tent_matrix(gx, VW)
            if not (A.any() and B.any() and (wz0 or wz1)):
                continue
            samples.append(dict(k=k, z0=min(max(z0, 0), VD - 1),
                                z1=min(max(z0 + 1, 0), VD - 1),
                                wz0=wz0, wz1=wz1, A=A, B=B))
        cams.append(samples)
    return cams


def _dedup(cams):
    """Group cameras with identical plans. Returns (unique_plans, cam2u)."""
    uniq, cam2u = [], []
    for s in cams:
        found = None
        for ui, u in enumerate(uniq):
            if len(u) == len(s) and all(
                a["k"] == b["k"] and a["z0"] == b["z0"] and a["z1"] == b["z1"]
                and a["wz0"] == b["wz0"] and a["wz1"] == b["wz1"]
                and np.array_equal(a["A"], b["A"]) and np.array_equal(a["B"], b["B"])
                for a, b in zip(u, s)
            ):
                found = ui
                break
        if found is None:
            uniq.append(list(s))
            cam2u.append(len(uniq) - 1)
        else:
            cam2u.append(found)
    return uniq, cam2u


def _pad_plans(uniq):
    """Pad every unique plan to a common NK with all-zero dummy samples
    (zero A/B/wz => exact zero density, raymarch unaffected)."""
    nk = max([len(u) for u in uniq] + [1])
    zero = dict(k=-1, z0=0, z1=0, wz0=np.float32(0), wz1=np.float32(0),
                A=np.zeros((PH, VH), np.float32),
                B=np.zeros((PW, VW), np.float32))
    for u in uniq:
        while len(u) < nk:
            u.append(zero)
    return nk


# ------------------------------------------------------------ device program
_PROG_CACHE = {}


def _build_program(NU, NK, WC, cam2u):
    import concourse.bacc as bacc
    import concourse.mybir as mybir
    import concourse.tile as tile

    F32 = mybir.dt.float32
    AF = mybir.ActivationFunctionType
    AX = mybir.AxisListType
    OP = mybir.AluOpType

    nc = bacc.Bacc(None)
    # head = at | bt | vol(k=0) ; vol2 = vol(k>0)  (fewer DMA issues)
    AT_COLS = NU * NK * PH
    BT_COLS = NU * NK * PCW
    V0_COLS = NU * CH * WC
    head_d = nc.dram_tensor("headpack", [128, AT_COLS + BT_COLS + V0_COLS], F32,
                            kind="ExternalInput")
    vol2_d = nc.dram_tensor("vol2pack", [128, max(1, NU * (NK - 1) * CH * WC)], F32,
                            kind="ExternalInput")
    tgt_d = nc.dram_tensor("tgtpack", [PH, N_CAM * 4 * PCW], F32,
                           kind="ExternalInput")
    bev_d = nc.dram_tensor("bevpack", [128, WB * VD], F32, kind="ExternalInput")
    out_d = nc.dram_tensor("out", [1, 16], F32, kind="ExternalOutput")

    with tile.TileContext(nc) as tc:
        with tc.tile_pool(name="sb", bufs=1) as sb, \
             tc.tile_pool(name="sb2", bufs=3) as sb2, \
             tc.tile_pool(name="ps1", bufs=3, space="PSUM") as ps1, \
             tc.tile_pool(name="ps2", bufs=3, space="PSUM") as ps2, \
             tc.tile_pool(name="ps3", bufs=1, space="PSUM") as ps3:

            head_sb = sb.tile([128, AT_COLS + BT_COLS + V0_COLS], F32)
            vol2_sb = sb.tile([128, max(1, NU * (NK - 1) * CH * WC)], F32)
            tgt_sb = sb.tile([PH, N_CAM * 4 * PCW], F32)
            bev_sb = sb.tile([128, WB * VD], F32)
            at_sb = head_sb[:, 0:AT_COLS]
            bt_sb = head_sb[0:WC, AT_COLS:AT_COLS + BT_COLS]

            # warm the ACT tables (Copy + Sqrt) while DMAs stream
            pack = sb.tile([128, 16], F32)
            nc.gpsimd.memset(pack[:], 0.0)
            ones = sb.tile([128, 1], F32)
            nc.gpsimd.memset(ones[:], 1.0)
            warm = sb.tile([1, 1], F32)
            nc.scalar.activation(warm[:], ones[0:1, :], AF.Sqrt,
                                 bias=1.0, scale=100.0)
            warm2 = sb.tile([1, 1], F32)
            nc.scalar.copy(out=warm2[:], in_=ones[0:1, :])
            warm_src = sb.tile([128, 512], F32)
            nc.vector.memset(warm_src[:], 1.0)
            wones = sb.tile([128, 1], F32)
            nc.vector.memset(wones[:], 1.0)
            warm_ps = ps3.tile([1, 512], F32, tag="warmps")
            for _ in range(3):
                nc.tensor.matmul(warm_ps[:], lhsT=wones[:], rhs=warm_src[:],
                                 start=True, stop=True)

            nc.sync.dma_start(out=head_sb[:], in_=head_d[:])
            if NK >= 3:
                c1 = CH * WC
                nc.sync.dma_start(out=vol2_sb[:, :c1], in_=vol2_d[:, :c1])
                nc.sync.dma_start(out=vol2_sb[:, c1:], in_=vol2_d[:, c1:])
            else:
                nc.sync.dma_start(out=vol2_sb[:], in_=vol2_d[:])
            nc.sync.dma_start(out=tgt_sb[:], in_=tgt_d[:])
            nc.sync.dma_start(out=bev_sb[:], in_=bev_d[:])

            def vol_slice(u, k, ch):
                if k == 0:
                    off = AT_COLS + BT_COLS + (u * CH + ch) * WC
                    return head_sb[:, off:off + WC]
                off = (u * (NK - 1) + (k - 1)) * CH * WC + ch * WC
                return vol2_sb[:, off:off + WC]

            # ---- render, software-pipelined on PE ----
            # step1(u,k): y_ps [WC, CH*PH] = 4 matmuls (per ch)
            # step2(u,k): p_ps [PH, CH*PCW] = 4 matmuls, transposed output
            #             (rows on partitions: cheap raymarch ops)
            y_sbs = {}
            p_ps_t = {}
            P = [[None] * NK for _ in range(NU)]

            def step1(u, k):
                uk = u * NK + k
                y_ps = ps1.tile([WC, CH * PH], F32, tag="y")
                aoff = uk * PH
                for ch in range(CH):
                    nc.tensor.matmul(
                        y_ps[:, ch * PH:(ch + 1) * PH],
                        lhsT=vol_slice(u, k, ch),
                        rhs=at_sb[:, aoff:aoff + PH],
                        start=True, stop=True)
                y_sb = sb2.tile([WC, CH * PH], F32, tag="ysb")
                nc.scalar.copy(out=y_sb[:], in_=y_ps[:])
                y_sbs[(u, k)] = y_sb

            def step2(u, k):
                uk = u * NK + k
                y_sb = y_sbs[(u, k)]
                p_ps = ps2.tile([PH, CH * PCW], F32, tag="p")
                for ch in range(CH):
                    nc.tensor.matmul(
                        p_ps[:, ch * PCW:(ch + 1) * PCW],
                        lhsT=y_sb[:, ch * PH:(ch + 1) * PH],
                        rhs=bt_sb[:, uk * PCW:(uk + 1) * PCW],
                        start=True, stop=True)
                p_sb = sb.tile([PH, CH * PCW], F32, tag=f"p_{u}_{k}")
                nc.scalar.copy(out=p_sb[:], in_=p_ps[:])
                P[u][k] = p_sb

            units = [(u, k) for u in range(NU) for k in range(NK)]
            step1(*units[0])
            for i in range(1, len(units)):
                step1(*units[i])
                step2(*units[i - 1])
            step2(*units[-1])

            # ---- raymarch per unique cam; rend[u] [PH, 4*PCW] matches tgt ----
            rend = [None] * NU
            for u in range(NU):
                r_t = sb.tile([PH, 4 * PCW], F32, tag=f"rend{u}")
                f3 = r_t[:, PCW:4 * PCW].rearrange("p (c f) -> p c f", c=3)
                d0 = P[u][0][:, 0:PCW]
                d0b = d0.rearrange("p (o f) -> p o f", o=1) \
                    .to_broadcast([PH, 3, PCW])
                rgb0 = P[u][0][:, PCW:4 * PCW].rearrange("p (c f) -> p c f", c=3)
                nc.vector.tensor_mul(f3, d0b, rgb0)
                shifted = sb.tile([PH, PCW], F32, tag=f"sh{u}_0")
                nc.vector.tensor_scalar(shifted[:], d0, -1.0, 1.0, OP.mult, OP.add)
                for k in range(1, NK):
                    dk = P[u][k][:, 0:PCW]
                    rgbk = P[u][k][:, PCW:4 * PCW].rearrange("p (c f) -> p c f", c=3)
                    w_t = sb.tile([PH, PCW], F32, tag=f"w{u}_{k}")
                    nc.vector.tensor_mul(w_t[:], dk, shifted[:])
                    wb = w_t[:].rearrange("p (o f) -> p o f", o=1) \
                        .to_broadcast([PH, 3, PCW])
                    t_t = sb.tile([PH, 3 * PCW], F32, tag=f"t{u}_{k}")
                    t3 = t_t[:].rearrange("p (c f) -> p c f", c=3)
                    nc.vector.tensor_mul(t3, wb, rgbk)
                    nc.vector.tensor_add(f3, f3, t3)
                    sh_new = sb.tile([PH, PCW], F32, tag=f"sh{u}_{k}")
                    # shifted*(1-dk) == shifted - shifted*dk == shifted - w
                    nc.vector.tensor_sub(sh_new[:], shifted[:], w_t[:])
                    shifted = sh_new
                nc.vector.tensor_scalar(r_t[:, 0:PCW], shifted[:], -1.0, 1.0,
                                        OP.mult, OP.add)
                rend[u] = r_t

            # ---- Huber losses per camera: accumulate sum(sqrt(1+100*d^2)) ----
            # pack cols: cam*2 = sil sum, cam*2+1 = color sum (3ch), 15 = bev
            for cam in range(N_CAM):
                u = cam2u[cam]
                diff = sb.tile([PH, 4 * PCW], F32, tag=f"diff{cam}")
                nc.vector.tensor_sub(diff[:], rend[u][:],
                                     tgt_sb[:, cam * 4 * PCW:(cam + 1) * 4 * PCW])
                sq = sb.tile([PH, 4 * PCW], F32, tag=f"sq{cam}")
                nc.vector.tensor_mul(sq[:], diff[:], diff[:])
                hub_s = sb.tile([PH, PCW], F32, tag=f"hub_s{cam}")
                nc.scalar.activation(hub_s[:], sq[:, 0:PCW], AF.Sqrt,
                                     bias=1.0, scale=100.0,
                                     accum_out=pack[0:PH, cam * 2:cam * 2 + 1])
                hub_c = sb.tile([PH, 3 * PCW], F32, tag=f"hub_c{cam}")
                nc.scalar.activation(hub_c[:], sq[:, PCW:4 * PCW], AF.Sqrt,
                                     bias=1.0, scale=100.0,
                                     accum_out=pack[0:PH, cam * 2 + 1:cam * 2 + 2])

            # ---- bev: sum over (h,w-chunk) of |max_d density| (2 halves) ----
            bmax = sb.tile([128, WB], F32)
            hw = WB // 2
            for h in range(2):
                nc.vector.reduce_max(
                    bmax[:, h * hw:(h + 1) * hw],
                    bev_sb[:, h * hw * VD:(h + 1) * hw * VD]
                    .rearrange("p (w d) -> p w d", d=VD),
                    axis=AX.X)
            nc.vector.tensor_reduce(pack[:, 15:16], bmax[:], axis=AX.X, op=OP.add,
                                    apply_absolute_value=True)

            # ---- cross-partition reduction via ones-matmul ----
            out_ps = ps3.tile([1, 16], F32)
            nc.tensor.matmul(out_ps[:], lhsT=ones[:], rhs=pack[:],
                             start=True, stop=True)
            out_sb = sb.tile([1, 16], F32)
            nc.scalar.copy(out=out_sb[:], in_=out_ps[:])
            nc.sync.dma_start(out=out_d[:], in_=out_sb[:])

    nc.compile()
    return nc


# ------------------------------------------------------------- host packing
def _pack_core(core, uniq, NK, WC, vol, dens, tsil, timg):
    NU = len(uniq)
    qlo = core * PCW
    qhi = qlo + PCW
    # union W-range over all (u,k) for this core's pixel columns
    wlo = VW
    whi = 0
    for u in uniq:
        for s in u:
            cols = np.nonzero(s["B"][qlo:qhi].any(axis=0))[0]
            if cols.size:
                wlo = min(wlo, int(cols[0]))
                whi = max(whi, int(cols[-1]) + 1)
    if wlo >= whi:
        wlo, whi = 0, 1
    if whi - wlo > WC:
        raise AssertionError(f"core {core}: W-range {whi - wlo} > WC {WC}")
    wlo = min(wlo, VW - 1)
    span = min(WC, VW - wlo)

    AT_COLS = NU * NK * PH
    BT_COLS = NU * NK * PCW
    V0_COLS = NU * CH * WC
    headpack = np.zeros((128, AT_COLS + BT_COLS + V0_COLS), np.float32)
    vol2pack = np.zeros((128, max(1, NU * (NK - 1) * CH * WC)), np.float32)
    for ui, u in enumerate(uniq):
        for k, s in enumerate(u):
            uk = ui * NK + k
            # z-preblended volume chunk for this (u,k)
            for ch in range(CH):
                blend = (s["wz0"] * vol[ch, s["z0"], :, wlo:wlo + span]
                         + s["wz1"] * vol[ch, s["z1"], :, wlo:wlo + span]) \
                    .astype(np.float32)
                if k == 0:
                    off = AT_COLS + BT_COLS + (ui * CH + ch) * WC
                    headpack[:, off:off + span] = blend
                else:
                    off = (ui * (NK - 1) + (k - 1)) * CH * WC + ch * WC
                    vol2pack[:, off:off + span] = blend
            headpack[:, uk * PH:(uk + 1) * PH] = s["A"].T
            headpack[:span, AT_COLS + uk * PCW:AT_COLS + (uk + 1) * PCW] = \
                s["B"][qlo:qhi, wlo:wlo + span].T
    tgtpack = np.zeros((PH, N_CAM * 4 * PCW), np.float32)
    for cam in range(N_CAM):
        tgtpack[:, (cam * 4) * PCW:(cam * 4 + 1) * PCW] = tsil[cam, :, qlo:qhi]
        for c in range(3):
            tgtpack[:, (cam * 4 + 1 + c) * PCW:(cam * 4 + 2 + c) * PCW] = \
                timg[cam, :, qlo:qhi, c]
    blo = core * WB
    bevpack = np.ascontiguousarray(
        dens[:, :, blo:blo + WB].transpose(1, 2, 0)).reshape(128, WB * VD)
    return dict(headpack=headpack, vol2pack=vol2pack,
                tgtpack=tgtpack, bevpack=np.ascontiguousarray(bevpack))


def _compute_wc(uniq):
    """Max W-range width over all cores, padded to a multiple of 4."""
    wc = 1
    for core in range(N_CORES):
        qlo, qhi = core * PCW, (core + 1) * PCW
        wlo, whi = VW, 0
        for u in uniq:
            for s in u:
                cols = np.nonzero(s["B"][qlo:qhi].any(axis=0))[0]
                if cols.size:
                    wlo = min(wlo, int(cols[0]))
                    whi = max(whi, int(cols[-1]) + 1)
        if wlo < whi:
            wc = max(wc, whi - wlo)
    wc = min(-(-wc // 4) * 4, 128)
    return wc


# ------------------------------------------------------------------- kernel
_RUN_MODE = "hw"     # "hw" | "sim" (CoreSim, debugging only)


def _run(nc, in_maps):
    if _RUN_MODE == "sim":
        from concourse.bass_interp import CoreSim

        class R:
            results = []
        for m in in_maps:
            sim = CoreSim(nc)
            for name, arr in m.items():
                sim.tensor(name)[:] = arr
            sim.simulate()
            R.results.append({"out": np.array(sim.tensor("out"))})
        return R
    from concourse.bass_utils import run_bass_kernel_spmd
    res = run_bass_kernel_spmd(nc, in_maps, list(range(N_CORES)))
    global _LAST_RESULT
    _LAST_RESULT = res
    return res


_LAST_RESULT = None


def kernel(densities, colors, target_silhouettes, target_images,
           focal, principal, R, T):

    densities = np.asarray(densities, np.float32)
    colors = np.asarray(colors, np.float32)
    tsil = np.asarray(target_silhouettes, np.float32)
    timg = np.asarray(target_images, np.float32)

    cams = _plan(focal, principal, R, T)
    uniq, cam2u = _dedup(cams)
    NK = _pad_plans(uniq)
    NU = len(uniq)
    WC = _compute_wc(uniq)
    assert WC <= 128, f"WC={WC} exceeds PE stationary width"

    key = (NU, NK, WC, tuple(cam2u))
    if key not in _PROG_CACHE:
        _PROG_CACHE[key] = _build_program(NU, NK, WC, cam2u)
    nc = _PROG_CACHE[key]

    vol = np.concatenate([densities[0], colors[0]], axis=0)  # [4,VD,VH,VW]
    dens = densities[0, 0]                                    # [VD,VH,VW]
    in_maps = [_pack_core(c, uniq, NK, WC, vol, dens, tsil, timg)
               for c in range(N_CORES)]
    res = _run(nc, in_maps)

    sil_S = 0.0
    col_S = 0.0
    bev_S = 0.0
    for c in range(N_CORES):
        o = res.results[c]["out"][0]
        for cam in range(N_CAM):
            sil_S += float(o[cam * 2])
            col_S += float(o[cam * 2 + 1])
        bev_S += float(o[15])
    n_sil = N_CAM * PH * PW
    n_col = N_CAM * PH * PW * 3
    sil_err = np.float32(0.1 * (sil_S - n_sil) / n_sil)
    col_err = np.float32(0.1 * (col_S - n_col) / n_col)
    bev_err = np.float32(bev_S / (VH * VW))
    return (col_err, sil_err, bev_err)


# revision 26
# speedup vs baseline: 1.1732x; 1.0748x over previous
"""Trainium2 Bass kernel for nn_DifferentialRenderLoss.

Algorithm: the volume-render trilinear gather is separable per depth sample
(rays are axis-aligned: R == I).  For depth sample k the rendered frame is
  out = A_k @ ((1-fz) vol[z0] + fz vol[z1]) @ B_k^T
with A_k [PH,VH], B_k [PW,VW] sparse tent-weight matrices (<=2 nnz/row).
Only samples whose z lies inside the volume contribute (~4 of 200); the
rest multiply exact 1.0 factors into the raymarch and add exact 0.0 to the
sums, so skipping them is lossless.  The z-blend folds into A (two
PSUM-accumulated matmuls).  Each of the 8 cores renders an 18-pixel-wide
column stripe of all cameras (loading only the volume W-slices it needs),
does the raymarch + Huber losses for its stripe, plus a W-chunk of the BEV
reduction, and writes partial sums; the host combines them.
"""
import sys

if "/opt/trn_rl_repo" not in sys.path:
    sys.path.insert(0, "/opt/trn_rl_repo")

import numpy as np

# ---- problem configuration (mirrors the nn.Module init_kwargs) ----
N_CAM = 2
PH, PW = 96, 144
NPTS = 200
MIN_DEPTH, MAX_DEPTH = 1.0, 4000.0
VD, VH, VW = 32, 128, 384
VOXEL = 2.5
VOL_TRANS = np.zeros(3, np.float32)
CH = 4                      # density + rgb
N_CORES = 8
PCW = PW // N_CORES         # pixel columns per core
WB = VW // N_CORES          # bev W-chunk per core


# ---------------------------------------------------------------- host math
def _tent_matrix(g, n):
    """Dense interpolation matrix mirroring the reference's floor/frac +
    per-corner mask + clip arithmetic bitwise (all float32)."""
    P = g.shape[0]
    A = np.zeros((P, n), np.float32)
    g0 = np.floor(g)
    f = (g - g0).astype(np.float32)
    i0 = g0.astype(np.int32)
    rows = np.arange(P)
    for d, w in ((0, (np.float32(1.0) - f).astype(np.float32)), (1, f)):
        idx = i0 + d
        valid = (idx >= 0) & (idx < n)
        np.add.at(A, (rows, np.clip(idx, 0, n - 1)),
                  np.where(valid, w, np.float32(0.0)).astype(np.float32))
    return A


def _plan(focal, principal, R, T):
    """Per-camera active depth samples with tent matrices (float32 host math
    mirroring the reference)."""
    focal = np.asarray(focal, np.float32)
    principal = np.asarray(principal, np.float32)
    R = np.asarray(R, np.float32)
    T = np.asarray(T, np.float32)
    xs = np.arange(PW, dtype=np.float32) + np.float32(0.5)
    ys = np.arange(PH, dtype=np.float32) + np.float32(0.5)
    depths = np.linspace(MIN_DEPTH, MAX_DEPTH, NPTS, dtype=np.float32)
    half = np.array([VOXEL * (VW - 1) / 2.0, VOXEL * (VH - 1) / 2.0,
                     VOXEL * (VD - 1) / 2.0], np.float32)
    cams = []
    for c in range(N_CAM):
        if not np.allclose(R[c], np.eye(3), atol=1e-6):
            raise NotImplementedError("kernel fast path requires R == I")
        dir_x = ((xs - principal[c, 0]) / focal[c, 0]).astype(np.float32)
        dir_y = ((ys - principal[c, 1]) / focal[c, 1]).astype(np.float32)
        origin = (-(T[c] @ R[c].T)).astype(np.float32)
        samples = []
        for k in range(NPTS):
            t = depths[k]
            zw = np.float32(origin[2] + t)          # dir_z == 1
            gz = np.float32((np.float32((zw - VOL_TRANS[2]) / half[2]) + np.float32(1.0))
                            * np.float32(0.5) * (VD - 1))
            if gz <= -1.0 or gz >= VD:
                continue
            z0 = int(np.floor(gz))
            fz = np.float32(gz - np.floor(gz))
            wz0 = np.float32(1.0) - fz if z0 >= 0 else np.float32(0.0)
            wz1 = fz if z0 + 1 <= VD - 1 else np.float32(0.0)
            gy = ((((origin[1] + t * dir_y) - VOL_TRANS[1]) / half[1]
                   + np.float32(1.0)) * np.float32(0.5) * (VH - 1)).astype(np.float32)
            gx = ((((origin[0] + t * dir_x) - VOL_TRANS[0]) / half[0]
                   + np.float32(1.0)) * np.float32(0.5) * (VW - 1)).astype(np.float32)
            A = _tent_matrix(gy, VH)
            B = _tent_matrix(gx, VW)
            if not (A.any() and B.any() and (wz0 or wz1)):
                continue
            samples.append(dict(k=k, z0=min(max(z0, 0), VD - 1),
                                z1=min(max(z0 + 1, 0), VD - 1),
                                wz0=wz0, wz1=wz1, A=A, B=B))
        cams.append(samples)
    return cams


def _dedup(cams):
    """Group cameras with identical plans. Returns (unique_plans, cam2u)."""
    uniq, cam2u = [], []
    for s in cams:
        found = None
        for ui, u in enumerate(uniq):
            if len(u) == len(s) and all(
                a["k"] == b["k"] and a["z0"] == b["z0"] and a["z1"] == b["z1"]
                and a["wz0"] == b["wz0"] and a["wz1"] == b["wz1"]
                and np.array_equal(a["A"], b["A"]) and np.array_equal(a["B"], b["B"])
                for a, b in zip(u, s)
            ):
                found = ui
                break
        if found is None:
            uniq.append(list(s))
            cam2u.append(len(uniq) - 1)
        else:
            cam2u.append(found)
    return uniq, cam2u


def _pad_plans(uniq):
    """Pad every unique plan to a common NK with all-zero dummy samples
    (zero A/B/wz => exact zero density, raymarch unaffected)."""
    nk = max([len(u) for u in uniq] + [1])
    zero = dict(k=-1, z0=0, z1=0, wz0=np.float32(0), wz1=np.float32(0),
                A=np.zeros((PH, VH), np.float32),
                B=np.zeros((PW, VW), np.float32))
    for u in uniq:
        while len(u) < nk:
            u.append(zero)
    return nk


# ------------------------------------------------------------ device program
_PROG_CACHE = {}


def _build_program(NU, NK, WC, cam2u):
    import concourse.bacc as bacc
    import concourse.mybir as mybir
    import concourse.tile as tile

    F32 = mybir.dt.float32
    AF = mybir.ActivationFunctionType
    AX = mybir.AxisListType
    OP = mybir.AluOpType

    nc = bacc.Bacc(None)
    # head = at | bt | vol(k=0) ; vol2 = vol(k>0)  (fewer DMA issues)
    AT_COLS = NU * NK * PH
    BT_COLS = NU * NK * PCW
    V0_COLS = NU * CH * WC
    head_d = nc.dram_tensor("headpack", [128, AT_COLS + BT_COLS + V0_COLS], F32,
                            kind="ExternalInput")
    vol2_d = nc.dram_tensor("vol2pack", [128, max(1, NU * (NK - 1) * CH * WC)], F32,
                            kind="ExternalInput")
    tgt_d = nc.dram_tensor("tgtpack", [PH, N_CAM * 4 * PCW], F32,
                           kind="ExternalInput")
    bev_d = nc.dram_tensor("bevpack", [128, WB * VD], F32, kind="ExternalInput")
    out_d = nc.dram_tensor("out", [1, 16], F32, kind="ExternalOutput")

    with tile.TileContext(nc) as tc:
        with tc.tile_pool(name="sb", bufs=1) as sb, \
             tc.tile_pool(name="sb2", bufs=3) as sb2, \
             tc.tile_pool(name="ps1", bufs=3, space="PSUM") as ps1, \
             tc.tile_pool(name="ps2", bufs=3, space="PSUM") as ps2, \
             tc.tile_pool(name="ps3", bufs=1, space="PSUM") as ps3:

            head_sb = sb.tile([128, AT_COLS + BT_COLS + V0_COLS], F32)
            vol2_sb = sb.tile([128, max(1, NU * (NK - 1) * CH * WC)], F32)
            tgt_sb = sb.tile([PH, N_CAM * 4 * PCW], F32)
            bev_sb = sb.tile([128, WB * VD], F32)
            at_sb = head_sb[:, 0:AT_COLS]
            bt_sb = head_sb[0:WC, AT_COLS:AT_COLS + BT_COLS]

            # warm the ACT tables (Copy + Sqrt) while DMAs stream
            pack = sb.tile([128, 16], F32)
            nc.gpsimd.memset(pack[:], 0.0)
            ones = sb.tile([128, 1], F32)
            nc.gpsimd.memset(ones[:], 1.0)
            warm = sb.tile([1, 1], F32)
            nc.scalar.activation(warm[:], ones[0:1, :], AF.Sqrt,
                                 bias=1.0, scale=100.0)
            warm2 = sb.tile([1, 1], F32)
            nc.scalar.copy(out=warm2[:], in_=ones[0:1, :])
            warm_src = sb.tile([128, 512], F32)
            nc.vector.memset(warm_src[:], 1.0)
            wones = sb.tile([128, 1], F32)
            nc.vector.memset(wones[:], 1.0)
            warm_ps = ps3.tile([1, 512], F32, tag="warmps")
            for _ in range(2):
                nc.tensor.matmul(warm_ps[:], lhsT=wones[:], rhs=warm_src[:],
                                 start=True, stop=True)

            nc.sync.dma_start(out=head_sb[:], in_=head_d[:])
            if NK >= 3:
                c1 = CH * WC
                nc.sync.dma_start(out=vol2_sb[:, :c1], in_=vol2_d[:, :c1])
                nc.sync.dma_start(out=vol2_sb[:, c1:], in_=vol2_d[:, c1:])
            else:
                nc.sync.dma_start(out=vol2_sb[:], in_=vol2_d[:])
            nc.sync.dma_start(out=tgt_sb[:], in_=tgt_d[:])
            nc.sync.dma_start(out=bev_sb[:], in_=bev_d[:])

            def vol_slice(u, k, ch):
                if k == 0:
                    off = AT_COLS + BT_COLS + (u * CH + ch) * WC
                    return head_sb[:, off:off + WC]
                off = (u * (NK - 1) + (k - 1)) * CH * WC + ch * WC
                return vol2_sb[:, off:off + WC]

            # ---- render, software-pipelined on PE ----
            # step1(u,k): y_ps [WC, CH*PH] = 4 matmuls (per ch)
            # step2(u,k): p_ps [PH, CH*PCW] = 4 matmuls, transposed output
            #             (rows on partitions: cheap raymarch ops)
            y_sbs = {}
            p_ps_t = {}
            P = [[None] * NK for _ in range(NU)]

            def step1(u, k):
                uk = u * NK + k
                y_ps = ps1.tile([WC, CH * PH], F32, tag="y")
                aoff = uk * PH
                for ch in range(CH):
                    nc.tensor.matmul(
                        y_ps[:, ch * PH:(ch + 1) * PH],
                        lhsT=vol_slice(u, k, ch),
                        rhs=at_sb[:, aoff:aoff + PH],
                        start=True, stop=True)
                y_sb = sb2.tile([WC, CH * PH], F32, tag="ysb")
                nc.scalar.copy(out=y_sb[:], in_=y_ps[:])
                y_sbs[(u, k)] = y_sb

            def step2(u, k):
                uk = u * NK + k
                y_sb = y_sbs[(u, k)]
                p_ps = ps2.tile([PH, CH * PCW], F32, tag="p")
                for ch in range(CH):
                    nc.tensor.matmul(
                        p_ps[:, ch * PCW:(ch + 1) * PCW],
                        lhsT=y_sb[:, ch * PH:(ch + 1) * PH],
                        rhs=bt_sb[:, uk * PCW:(uk + 1) * PCW],
                        start=True, stop=True)
                p_sb = sb.tile([PH, CH * PCW], F32, tag=f"p_{u}_{k}")
                nc.scalar.copy(out=p_sb[:], in_=p_ps[:])
                P[u][k] = p_sb

            units = [(u, k) for u in range(NU) for k in range(NK)]
            step1(*units[0])
            for i in range(1, len(units)):
                step1(*units[i])
                step2(*units[i - 1])
            step2(*units[-1])

            # ---- raymarch per unique cam; rend[u] [PH, 4*PCW] matches tgt ----
            rend = [None] * NU
            for u in range(NU):
                r_t = sb.tile([PH, 4 * PCW], F32, tag=f"rend{u}")
                f3 = r_t[:, PCW:4 * PCW].rearrange("p (c f) -> p c f", c=3)
                d0 = P[u][0][:, 0:PCW]
                d0b = d0.rearrange("p (o f) -> p o f", o=1) \
                    .to_broadcast([PH, 3, PCW])
                rgb0 = P[u][0][:, PCW:4 * PCW].rearrange("p (c f) -> p c f", c=3)
                nc.vector.tensor_mul(f3, d0b, rgb0)
                shifted = sb.tile([PH, PCW], F32, tag=f"sh{u}_0")
                nc.vector.tensor_scalar(shifted[:], d0, -1.0, 1.0, OP.mult, OP.add)
                for k in range(1, NK):
                    dk = P[u][k][:, 0:PCW]
                    rgbk = P[u][k][:, PCW:4 * PCW].rearrange("p (c f) -> p c f", c=3)
                    w_t = sb.tile([PH, PCW], F32, tag=f"w{u}_{k}")
                    nc.vector.tensor_mul(w_t[:], dk, shifted[:])
                    wb = w_t[:].rearrange("p (o f) -> p o f", o=1) \
                        .to_broadcast([PH, 3, PCW])
                    t_t = sb.tile([PH, 3 * PCW], F32, tag=f"t{u}_{k}")
                    t3 = t_t[:].rearrange("p (c f) -> p c f", c=3)
                    nc.vector.tensor_mul(t3, wb, rgbk)
                    nc.vector.tensor_add(f3, f3, t3)
                    sh_new = sb.tile([PH, PCW], F32, tag=f"sh{u}_{k}")
                    # shifted*(1-dk) == shifted - shifted*dk == shifted - w
                    nc.vector.tensor_sub(sh_new[:], shifted[:], w_t[:])
                    shifted = sh_new
                nc.vector.tensor_scalar(r_t[:, 0:PCW], shifted[:], -1.0, 1.0,
                                        OP.mult, OP.add)
                rend[u] = r_t

            # ---- Huber losses per camera: accumulate sum(sqrt(1+100*d^2)) ----
            # pack cols: cam*2 = sil sum, cam*2+1 = color sum (3ch), 15 = bev
            for cam in range(N_CAM):
                u = cam2u[cam]
                diff = sb.tile([PH, 4 * PCW], F32, tag=f"diff{cam}")
                nc.vector.tensor_sub(diff[:], rend[u][:],
                                     tgt_sb[:, cam * 4 * PCW:(cam + 1) * 4 * PCW])
                sq = sb.tile([PH, 4 * PCW], F32, tag=f"sq{cam}")
                nc.vector.tensor_mul(sq[:], diff[:], diff[:])
                hub_s = sb.tile([PH, PCW], F32, tag=f"hub_s{cam}")
                nc.scalar.activation(hub_s[:], sq[:, 0:PCW], AF.Sqrt,
                                     bias=1.0, scale=100.0,
                                     accum_out=pack[0:PH, cam * 2:cam * 2 + 1])
                hub_c = sb.tile([PH, 3 * PCW], F32, tag=f"hub_c{cam}")
                nc.scalar.activation(hub_c[:], sq[:, PCW:4 * PCW], AF.Sqrt,
                                     bias=1.0, scale=100.0,
                                     accum_out=pack[0:PH, cam * 2 + 1:cam * 2 + 2])

            # ---- bev: sum over (h,w-chunk) of |max_d density| (2 halves) ----
            bmax = sb.tile([128, WB], F32)
            hw = WB // 2
            for h in range(2):
                nc.vector.reduce_max(
                    bmax[:, h * hw:(h + 1) * hw],
                    bev_sb[:, h * hw * VD:(h + 1) * hw * VD]
                    .rearrange("p (w d) -> p w d", d=VD),
                    axis=AX.X)
            nc.vector.tensor_reduce(pack[:, 15:16], bmax[:], axis=AX.X, op=OP.add,
                                    apply_absolute_value=True)

            # ---- cross-partition reduction via ones-matmul ----
            out_ps = ps3.tile([1, 16], F32)
            nc.tensor.matmul(out_ps[:], lhsT=ones[:], rhs=pack[:],
                             start=True, stop=True)
            out_sb = sb.tile([1, 16], F32)
            nc.scalar.copy(out=out_sb[:], in_=out_ps[:])
            nc.sync.dma_start(out=out_d[:], in_=out_sb[:])

    nc.compile()
    return nc


# ------------------------------------------------------------- host packing
def _pack_core(core, uniq, NK, WC, vol, dens, tsil, timg):
    NU = len(uniq)
    qlo = core * PCW
    qhi = qlo + PCW
    # union W-range over all (u,k) for this core's pixel columns
    wlo = VW
    whi = 0
    for u in uniq:
        for s in u:
            cols = np.nonzero(s["B"][qlo:qhi].any(axis=0))[0]
            if cols.size:
                wlo = min(wlo, int(cols[0]))
                whi = max(whi, int(cols[-1]) + 1)
    if wlo >= whi:
        wlo, whi = 0, 1
    if whi - wlo > WC:
        raise AssertionError(f"core {core}: W-range {whi - wlo} > WC {WC}")
    wlo = min(wlo, VW - 1)
    span = min(WC, VW - wlo)

    AT_COLS = NU * NK * PH
    BT_COLS = NU * NK * PCW
    V0_COLS = NU * CH * WC
    headpack = np.zeros((128, AT_COLS + BT_COLS + V0_COLS), np.float32)
    vol2pack = np.zeros((128, max(1, NU * (NK - 1) * CH * WC)), np.float32)
    for ui, u in enumerate(uniq):
        for k, s in enumerate(u):
            uk = ui * NK + k
            # z-preblended volume chunk for this (u,k)
            for ch in range(CH):
                blend = (s["wz0"] * vol[ch, s["z0"], :, wlo:wlo + span]
                         + s["wz1"] * vol[ch, s["z1"], :, wlo:wlo + span]) \
                    .astype(np.float32)
                if k == 0:
                    off = AT_COLS + BT_COLS + (ui * CH + ch) * WC
                    headpack[:, off:off + span] = blend
                else:
                    off = (ui * (NK - 1) + (k - 1)) * CH * WC + ch * WC
                    vol2pack[:, off:off + span] = blend
            headpack[:, uk * PH:(uk + 1) * PH] = s["A"].T
            headpack[:span, AT_COLS + uk * PCW:AT_COLS + (uk + 1) * PCW] = \
                s["B"][qlo:qhi, wlo:wlo + span].T
    tgtpack = np.zeros((PH, N_CAM * 4 * PCW), np.float32)
    for cam in range(N_CAM):
        tgtpack[:, (cam * 4) * PCW:(cam * 4 + 1) * PCW] = tsil[cam, :, qlo:qhi]
        for c in range(3):
            tgtpack[:, (cam * 4 + 1 + c) * PCW:(cam * 4 + 2 + c) * PCW] = \
                timg[cam, :, qlo:qhi, c]
    blo = core * WB
    bevpack = np.ascontiguousarray(
        dens[:, :, blo:blo + WB].transpose(1, 2, 0)).reshape(128, WB * VD)
    return dict(headpack=headpack, vol2pack=vol2pack,
                tgtpack=tgtpack, bevpack=np.ascontiguousarray(bevpack))


def _compute_wc(uniq):
    """Max W-range width over all cores, padded to a multiple of 4."""
    wc = 1
    for core in range(N_CORES):
        qlo, qhi = core * PCW, (core + 1) * PCW
        wlo, whi = VW, 0
        for u in uniq:
            for s in u:
                cols = np.nonzero(s["B"][qlo:qhi].any(axis=0))[0]
                if cols.size:
                    wlo = min(wlo, int(cols[0]))
                    whi = max(whi, int(cols[-1]) + 1)
        if wlo < whi:
            wc = max(wc, whi - wlo)
    wc = min(-(-wc // 4) * 4, 128)
    return wc


# ------------------------------------------------------------------- kernel
_RUN_MODE = "hw"     # "hw" | "sim" (CoreSim, debugging only)


def _run(nc, in_maps):
    if _RUN_MODE == "sim":
        from concourse.bass_interp import CoreSim

        class R:
            results = []
        for m in in_maps:
            sim = CoreSim(nc)
            for name, arr in m.items():
                sim.tensor(name)[:] = arr
            sim.simulate()
            R.results.append({"out": np.array(sim.tensor("out"))})
        return R
    from concourse.bass_utils import run_bass_kernel_spmd
    res = run_bass_kernel_spmd(nc, in_maps, list(range(N_CORES)))
    global _LAST_RESULT
    _LAST_RESULT = res
    return res


_LAST_RESULT = None


def kernel(densities, colors, target_silhouettes, target_images,
           focal, principal, R, T):

    densities = np.asarray(densities, np.float32)
    colors = np.asarray(colors, np.float32)
    tsil = np.asarray(target_silhouettes, np.float32)
    timg = np.asarray(target_images, np.float32)

    cams = _plan(focal, principal, R, T)
    uniq, cam2u = _dedup(cams)
    NK = _pad_plans(uniq)
    NU = len(uniq)
    WC = _compute_wc(uniq)
    assert WC <= 128, f"WC={WC} exceeds PE stationary width"

    key = (NU, NK, WC, tuple(cam2u))
    if key not in _PROG_CACHE:
        _PROG_CACHE[key] = _build_program(NU, NK, WC, cam2u)
    nc = _PROG_CACHE[key]

    vol = np.concatenate([densities[0], colors[0]], axis=0)  # [4,VD,VH,VW]
    dens = densities[0, 0]                                    # [VD,VH,VW]
    in_maps = [_pack_core(c, uniq, NK, WC, vol, dens, tsil, timg)
               for c in range(N_CORES)]
    res = _run(nc, in_maps)

    sil_S = 0.0
    col_S = 0.0
    bev_S = 0.0
    for c in range(N_CORES):
        o = res.results[c]["out"][0]
        for cam in range(N_CAM):
            sil_S += float(o[cam * 2])
            col_S += float(o[cam * 2 + 1])
        bev_S += float(o[15])
    n_sil = N_CAM * PH * PW
    n_col = N_CAM * PH * PW * 3
    sil_err = np.float32(0.1 * (sil_S - n_sil) / n_sil)
    col_err = np.float32(0.1 * (col_S - n_col) / n_col)
    bev_err = np.float32(bev_S / (VH * VW))
    return (col_err, sil_err, bev_err)
